# revision 18
# baseline (speedup 1.0000x reference)
"""v3 kernel: consumption-ordered startup DMA across 4 queues, PE warmup
matmuls, bf16 output, +bo on scalar engine (Identity+bias), finer tail
chunks fanned across queues.

Math: out = cs * (Wc x + bc) + bo with Wc = Wo Wv, bc = Wo bv,
cs = 1 + S/(L-1) - t*L/(L-1), t = sigmoid(log(L-1) - d),
d = x^T M x + u.x + c0, M = Wq^T Wk, u = Wk^T bq + Wq^T bk, c0 = bq.bk.
"""

import numpy as np
import ml_dtypes

import concourse.bass as bass  # noqa: F401
import concourse.tile as tile
from concourse import bacc, mybir
from concourse.bass_utils import run_bass_kernel_spmd

dt = mybir.dt
AF = mybir.ActivationFunctionType
ALU = mybir.AluOpType

N, L, H = 8, 2048, 1024
P = 128
LB = 512
NH = H // P
NL = L // LB
N_CORES = 8
NC = NH + 1 + NH + NH

_CACHE = {}


def _build():
    nc = bacc.Bacc("TRN2", target_bir_lowering=False, debug=False,
                   num_devices=N_CORES)

    xT_d = nc.dram_tensor("xT", [H, L], dt.bfloat16, kind="ExternalInput").ap()
    MT_d = nc.dram_tensor("MT", [NH, P, NH * P], dt.bfloat16,
                          kind="ExternalInput").ap()
    WcT_d = nc.dram_tensor("WcT", [NH, P, NH * P], dt.bfloat16,
                           kind="ExternalInput").ap()
    cp_d = nc.dram_tensor("cpack", [P, NC], dt.float32, kind="ExternalInput").ap()
    ones_d = nc.dram_tensor("ones", [P, P], dt.float32r,
                            kind="ExternalInput").ap()
    out_d = nc.dram_tensor("outT", [H, L], dt.bfloat16, kind="ExternalOutput").ap()

    xT3 = xT_d.rearrange("(j p) l -> p j l", p=P)

    with tile.TileContext(nc) as tc:
        with (
            tc.tile_pool(name="resident", bufs=1) as rp,
            tc.tile_pool(name="weights", bufs=1) as wtp,
            tc.tile_pool(name="work", bufs=3) as wp,
            tc.tile_pool(name="outwork", bufs=6) as op,
            tc.tile_pool(name="mmpsum", bufs=4, space="PSUM") as yp,
            tc.tile_pool(name="dpsum", bufs=2, space="PSUM") as dp,
            tc.tile_pool(name="wupsum", bufs=1, space="PSUM") as wup,
        ):
            t_s = rp.tile([P, L], dt.float32)
            cs = rp.tile([P, L], dt.float32)

            # --- PE warmup: junk matmuls to ride out the p-state ramp while
            # the first input DMAs are in flight.
            junk = rp.tile([P, LB], dt.bfloat16)
            nc.vector.memset(junk[:], 0.0)
            wpsum = wup.tile([P, LB], dt.float32)
            NWARM = 20
            for i in range(NWARM):
                nc.tensor.matmul(wpsum[:], junk[:, 0:P], junk[:],
                                 start=(i == 0), stop=(i == NWARM - 1))

            # --- startup DMA schedule: consumption order, round-robin over
            # 4 issue queues (sync, scalar, gpsimd, vector).
            mt = [None] * NH          # mt[ob]: [P, NH*P] (ob>=1)
            mt0c = [None] * 4         # mt0 in 4 chunks of 2 hb-blocks
            wct = [None] * NH
            xbt = {}                  # (lb, hb) -> AP [P, LB]

            def load_xb1(lb, hb, eng):
                t = wtp.tile([P, LB], dt.bfloat16, tag=f"xb{lb}_{hb}")
                eng.dma_start(t[:], xT3[:, hb, lb * LB:(lb + 1) * LB])
                xbt[(lb, hb)] = t[:]

            def load_xb4(lb, hb0, eng):
                t = wtp.tile([P, 4, LB], dt.bfloat16, tag=f"xb{lb}_{hb0}b")
                eng.dma_start(t[:], xT3[:, hb0:hb0 + 4,
                                        lb * LB:(lb + 1) * LB])
                for k in range(4):
                    xbt[(lb, hb0 + k)] = t[:, k, :]

            def load_mt0c(k, eng):
                t = wtp.tile([P, 2 * P], dt.bfloat16, tag=f"mt0c{k}")
                eng.dma_start(t[:], MT_d[0, :, 2 * k * P:(2 * k + 2) * P])
                mt0c[k] = t

            def load_w(dst, src3, ob, tag, eng):
                t = wtp.tile([P, NH * P], dt.bfloat16, tag=f"{tag}{ob}")
                eng.dma_start(t[:], src3[ob])
                dst[ob] = t

            cp = rp.tile([P, NC], dt.float32)
            ones = rp.tile([P, P], dt.float32r)

            # first group: xb(0,*) + mt0 chunks + split mt1, interleaved in
            # consumption order so the first matmuls are fed as early as
            # possible.  x tiles on sync/gpsimd; weights on scalar (whose
            # first DMA trails its ACT_TABLE_LOAD but weights are needed
            # slightly later than x).  mt1 is split across scalar+sync so it
            # lands before group 1 starts.
            mthalf = {}

            def load_mth(ob, half, eng):
                t = wtp.tile([P, 4 * P], dt.bfloat16, tag=f"mt{ob}h{half}")
                eng.dma_start(t[:], MT_d[ob, :, half * 4 * P:(half + 1) * 4 * P])
                mthalf[(ob, half)] = t

            load_xb1(0, 0, nc.sync)
            load_mt0c(0, nc.scalar)
            load_xb1(0, 1, nc.gpsimd)
            load_xb1(0, 2, nc.sync)
            load_mt0c(1, nc.scalar)
            load_xb1(0, 3, nc.gpsimd)
            load_xb1(0, 4, nc.sync)
            load_mt0c(2, nc.scalar)
            load_xb1(0, 5, nc.gpsimd)
            load_mth(1, 0, nc.scalar)
            load_xb1(0, 6, nc.sync)
            load_xb1(0, 7, nc.gpsimd)
            load_mt0c(3, nc.scalar)
            load_mth(1, 1, nc.sync)
            # constants + remaining phase-1 weights in consumption order
            nc.gpsimd.dma_start(cp[:], cp_d[:])
            nc.scalar.dma_start(ones[:], ones_d[:])
            load_mth(2, 0, nc.scalar)
            load_mth(2, 1, nc.sync)
            load_w(mt, MT_d, 3, "mt", nc.gpsimd)
            load_w(mt, MT_d, 4, "mt", nc.scalar)
            load_w(mt, MT_d, 5, "mt", nc.sync)
            load_w(mt, MT_d, 6, "mt", nc.gpsimd)
            load_w(mt, MT_d, 7, "mt", nc.scalar)
            # x for lb1 (needed ~26us in), then phase-2 weights; xb2/xb3 are
            # issued later from scalar after the lb0/lb1 sigmoids.
            load_xb4(1, 0, nc.gpsimd)
            load_xb4(1, 4, nc.sync)
            load_w(wct, WcT_d, 0, "wct", nc.gpsimd)
            load_w(wct, WcT_d, 1, "wct", nc.sync)
            load_w(wct, WcT_d, 2, "wct", nc.gpsimd)
            load_w(wct, WcT_d, 3, "wct", nc.sync)
            load_w(wct, WcT_d, 4, "wct", nc.gpsimd)
            load_w(wct, WcT_d, 5, "wct", nc.sync)
            load_w(wct, WcT_d, 6, "wct", nc.gpsimd)
            load_w(wct, WcT_d, 7, "wct", nc.sync)

            ub = cp[:, :NH]
            c0b = cp[:, NH:NH + 1]
            bcb = cp[:, NH + 1:NH + 1 + NH]
            bob = cp[:, NH + 1 + NH:]

            def mt_ap(ob, hb):
                if ob == 0:
                    return mt0c[hb // 2][:, (hb % 2) * P:(hb % 2 + 1) * P]
                if (ob, hb // 4) in mthalf:
                    return mthalf[(ob, hb // 4)][:, (hb % 4) * P:(hb % 4 + 1) * P]
                return mt[ob][:, hb * P:(hb + 1) * P]

            sp = [rp.tile([P, 1], dt.float32, name=f"sp{i}", tag=f"sp{i}")
                  for i in range(NL)]

            state = {"pending": None}

            def flush_pending():
                if state["pending"] is None:
                    return
                pd_t, prod_t, lb = state["pending"]
                state["pending"] = None
                nc.tensor.matmul(pd_t[:], ones[:], prod_t[:],
                                 start=True, stop=True)
                ls = slice(lb * LB, (lb + 1) * LB)
                nc.scalar.activation(t_s[:, ls], pd_t[:], AF.Sigmoid,
                                     bias=c0b[:, 0:1], scale=-1.0,
                                     accum_out=sp[lb][:])
                # deferred x prefetch: issue on scalar after the sigmoid so
                # these transfers stay off the wire during the hot start.
                if lb == 0:
                    load_xb4(2, 0, nc.scalar)
                    load_xb4(2, 4, nc.scalar)
                elif lb == 1:
                    load_xb4(3, 0, nc.scalar)
                    load_xb4(3, 4, nc.scalar)

            # ---- phase 1: d and sigmoid path
            for lb in range(NL):
                pd = dp.tile([P, LB], dt.float32)
                acc = None
                for ob in range(NH):
                    py = yp.tile([P, LB], dt.float32, tag="mm")
                    for hb in range(NH):
                        nc.tensor.matmul(
                            py[:], mt_ap(ob, hb), xbt[(lb, hb)],
                            start=(hb == 0), stop=(hb == NH - 1))
                    if ob == 1:
                        flush_pending()
                    prod = wp.tile([P, LB], dt.float32r, tag="prod")
                    nc.vector.scalar_tensor_tensor(
                        prod[:], py[:], ub[:, ob:ob + 1], xbt[(lb, ob)],
                        op0=ALU.add, op1=ALU.mult)
                    if acc is None:
                        acc = prod
                    else:
                        nacc = wp.tile([P, LB], dt.float32r, tag="pacc")
                        nc.vector.tensor_tensor(nacc[:], acc[:], prod[:],
                                                op=ALU.add)
                        acc = nacc
                state["pending"] = (pd, acc, lb)

            # ---- phase 2: out = cs*(Wc x + bc) + bo
            OUT_ENG = [nc.sync, nc.gpsimd]
            oi = 0
            for lb in range(NL):
                for ob in range(NH):
                    if lb == NL - 1 and ob >= NH - 2:
                        nmm = 4
                    else:
                        nmm = 1
                    mw = LB // nmm
                    pzs = []
                    for ck in range(nmm):
                        pz = yp.tile([P, mw], dt.float32, tag="mm")
                        for hb in range(NH):
                            nc.tensor.matmul(
                                pz[:], wct[ob][:, hb * P:(hb + 1) * P],
                                xbt[(lb, hb)][:, ck * mw:(ck + 1) * mw],
                                start=(hb == 0), stop=(hb == NH - 1))
                        pzs.append(pz)
                    if lb == 0 and ob == 0:
                        flush_pending()
                        s01 = rp.tile([P, 1], dt.float32)
                        nc.vector.tensor_tensor(s01[:], sp[0][:], sp[1][:],
                                                op=ALU.add)
                        s23 = rp.tile([P, 1], dt.float32)
                        nc.vector.tensor_tensor(s23[:], sp[2][:], sp[3][:],
                                                op=ALU.add)
                        s_all = rp.tile([P, 1], dt.float32)
                        nc.vector.tensor_tensor(s_all[:], s01[:], s23[:],
                                                op=ALU.add)
                        S1_t = rp.tile([P, 1], dt.float32)
                        nc.vector.tensor_scalar(
                            S1_t[:], s_all[:], 1.0 / (L - 1), 1.0,
                            op0=ALU.mult, op1=ALU.add)
                        nc.vector.tensor_scalar(
                            cs[:], t_s[:], -float(L) / (L - 1), S1_t[:],
                            op0=ALU.mult, op1=ALU.add)
                    for ck in range(nmm):
                        lo = lb * LB + ck * mw
                        lsc = slice(lo, lo + mw)
                        zc = op.tile([P, mw], dt.float32, tag="zc")
                        nc.vector.scalar_tensor_tensor(
                            zc[:], pzs[ck][:], bcb[:, ob:ob + 1], cs[:, lsc],
                            op0=ALU.add, op1=ALU.mult)
                        ot = op.tile([P, mw], dt.bfloat16, tag="ot")
                        if oi % 2 == 1:
                            nc.vector.tensor_scalar_add(
                                ot[:], zc[:], bob[:, ob:ob + 1])
                        else:
                            nc.scalar.activation(ot[:], zc[:], AF.Identity,
                                                 bias=bob[:, ob:ob + 1])
                        # keep gpsimd's DMA queue out of the last few tiles:
                        # whichever engine issues the final DMAs pays a long
                        # queue DRAIN at TileContext exit.
                        if lb == NL - 1 and ob >= NH - 4:
                            eng = [nc.sync, nc.scalar][oi % 2]
                        else:
                            eng = OUT_ENG[oi % len(OUT_ENG)]
                        oi += 1
                        eng.dma_start(out_d[ob * P:(ob + 1) * P, lsc], ot[:])

    nc.compile()
    return nc


def _get_nc():
    if "nc" not in _CACHE:
        _CACHE["nc"] = _build()
    return _CACHE["nc"]


def _prep_inputs(x, Wq, bq, Wk, bk, Wv, bv, Wo, bo):
    f8 = np.float64
    bf = ml_dtypes.bfloat16
    M = (Wq.astype(f8).T @ Wk.astype(f8)).astype(np.float32)
    u = (Wk.astype(f8).T @ bq.astype(f8)
         + Wq.astype(f8).T @ bk.astype(f8)).astype(np.float32)
    c0 = np.float32(bq.astype(f8) @ bk.astype(f8))
    Wc = (Wo.astype(f8) @ Wv.astype(f8)).astype(np.float32)
    bc = (Wo.astype(f8) @ bv.astype(f8)).astype(np.float32)

    def _pack(WT):
        t = WT.reshape(NH, P, NH, P)
        return np.ascontiguousarray(
            t.transpose(2, 1, 0, 3).reshape(NH, P, NH * P).astype(bf))

    MT = _pack(M.T)
    WcT = _pack(Wc.T)
    ub = u.reshape(NH, P).T
    bcb = bc.reshape(NH, P).T
    bob = bo.astype(np.float32).reshape(NH, P).T
    c0b = np.full((P, 1), np.log(L - 1.0) - np.float64(c0), np.float32)
    cpack = np.ascontiguousarray(
        np.concatenate([ub, c0b, bcb, bob], axis=1).astype(np.float32))
    ones = np.ones((P, P), np.float32)

    shared = dict(MT=MT, WcT=WcT, cpack=cpack, ones=ones)
    in_maps = []
    for n in range(N_CORES):
        xT = np.ascontiguousarray(x[n].T.astype(bf))
        in_maps.append(dict(xT=xT, **shared))
    return in_maps


def kernel(x, Wq, bq, Wk, bk, Wv, bv, Wo, bo, _trace=False, _trace_kwargs=None):
    x, Wq, bq, Wk, bk, Wv, bv, Wo, bo = (
        np.asarray(a) for a in (x, Wq, bq, Wk, bk, Wv, bv, Wo, bo))
    nc = _get_nc()
    in_maps = _prep_inputs(x, Wq, bq, Wk, bk, Wv, bv, Wo, bo)
    res = run_bass_kernel_spmd(nc, in_maps, list(range(N_CORES)),
                               trace=_trace, **(_trace_kwargs or {}))
    out = np.empty((N, L, H), np.float32)
    for n in range(N_CORES):
        out[n] = res.results[n]["outT"].T
    if _trace:
        kernel.last_result = res
    return out


# revision 20
# speedup vs baseline: 1.0381x; 1.0381x over previous
"""v3 kernel: consumption-ordered startup DMA across 4 queues, PE warmup
matmuls, bf16 output, +bo on scalar engine (Identity+bias), finer tail
chunks fanned across queues.

Math: out = cs * (Wc x + bc) + bo with Wc = Wo Wv, bc = Wo bv,
cs = 1 + S/(L-1) - t*L/(L-1), t = sigmoid(log(L-1) - d),
d = x^T M x + u.x + c0, M = Wq^T Wk, u = Wk^T bq + Wq^T bk, c0 = bq.bk.
"""

import numpy as np
import ml_dtypes

import concourse.bass as bass  # noqa: F401
import concourse.tile as tile
from concourse import bacc, mybir
from concourse.bass_utils import run_bass_kernel_spmd

dt = mybir.dt
AF = mybir.ActivationFunctionType
ALU = mybir.AluOpType

N, L, H = 8, 2048, 1024
P = 128
LB = 512
NH = H // P
NL = L // LB
N_CORES = 8
NC = NH + 1 + NH + NH

_CACHE = {}


def _build():
    nc = bacc.Bacc("TRN2", target_bir_lowering=False, debug=False,
                   num_devices=N_CORES)

    xT_d = nc.dram_tensor("xT", [H, L], dt.bfloat16, kind="ExternalInput").ap()
    MT_d = nc.dram_tensor("MT", [NH, P, NH * P], dt.bfloat16,
                          kind="ExternalInput").ap()
    WcT_d = nc.dram_tensor("WcT", [NH, P, NH * P], dt.bfloat16,
                           kind="ExternalInput").ap()
    cp_d = nc.dram_tensor("cpack", [P, NC], dt.float32, kind="ExternalInput").ap()
    ones_d = nc.dram_tensor("ones", [P, P], dt.float32r,
                            kind="ExternalInput").ap()
    out_d = nc.dram_tensor("outT", [H, L], dt.bfloat16, kind="ExternalOutput").ap()

    xT3 = xT_d.rearrange("(j p) l -> p j l", p=P)

    with tile.TileContext(nc) as tc:
        with (
            tc.tile_pool(name="resident", bufs=1) as rp,
            tc.tile_pool(name="weights", bufs=1) as wtp,
            tc.tile_pool(name="work", bufs=3) as wp,
            tc.tile_pool(name="outwork", bufs=6) as op,
            tc.tile_pool(name="mmpsum", bufs=4, space="PSUM") as yp,
            tc.tile_pool(name="dpsum", bufs=2, space="PSUM") as dp,
            tc.tile_pool(name="wupsum", bufs=1, space="PSUM") as wup,
        ):
            t_s = rp.tile([P, L], dt.float32)
            cs = rp.tile([P, L], dt.float32)

            # --- PE warmup: junk matmuls to ride out the p-state ramp while
            # the first input DMAs are in flight.
            junk = rp.tile([P, LB], dt.bfloat16)
            nc.vector.memset(junk[:], 0.0)
            wpsum = wup.tile([P, LB], dt.float32)
            NWARM = 16
            for i in range(NWARM):
                nc.tensor.matmul(wpsum[:], junk[:, 0:P], junk[:],
                                 start=(i == 0), stop=(i == NWARM - 1))

            # --- startup DMA schedule: consumption order, round-robin over
            # 4 issue queues (sync, scalar, gpsimd, vector).
            mt = [None] * NH          # mt[ob]: [P, NH*P] (ob>=1)
            mt0c = [None] * 4         # mt0 in 4 chunks of 2 hb-blocks
            wct = [None] * NH
            xbt = {}                  # (lb, hb) -> AP [P, LB]

            def load_xb1(lb, hb, eng):
                t = wtp.tile([P, LB], dt.bfloat16, tag=f"xb{lb}_{hb}")
                eng.dma_start(t[:], xT3[:, hb, lb * LB:(lb + 1) * LB])
                xbt[(lb, hb)] = t[:]

            def load_xb4(lb, hb0, eng):
                t = wtp.tile([P, 4, LB], dt.bfloat16, tag=f"xb{lb}_{hb0}b")
                eng.dma_start(t[:], xT3[:, hb0:hb0 + 4,
                                        lb * LB:(lb + 1) * LB])
                for k in range(4):
                    xbt[(lb, hb0 + k)] = t[:, k, :]

            def load_mt0c(k, eng):
                t = wtp.tile([P, 2 * P], dt.bfloat16, tag=f"mt0c{k}")
                eng.dma_start(t[:], MT_d[0, :, 2 * k * P:(2 * k + 2) * P])
                mt0c[k] = t

            def load_w(dst, src3, ob, tag, eng):
                t = wtp.tile([P, NH * P], dt.bfloat16, tag=f"{tag}{ob}")
                eng.dma_start(t[:], src3[ob])
                dst[ob] = t

            cp = rp.tile([P, NC], dt.float32)
            ones = rp.tile([P, P], dt.float32r)

            # first group: xb(0,*) + mt0 chunks + split mt1, interleaved in
            # consumption order so the first matmuls are fed as early as
            # possible.  x tiles on sync/gpsimd; weights on scalar (whose
            # first DMA trails its ACT_TABLE_LOAD but weights are needed
            # slightly later than x).  mt1 is split across scalar+sync so it
            # lands before group 1 starts.
            mthalf = {}

            def load_mth(ob, half, eng):
                t = wtp.tile([P, 4 * P], dt.bfloat16, tag=f"mt{ob}h{half}")
                eng.dma_start(t[:], MT_d[ob, :, half * 4 * P:(half + 1) * 4 * P])
                mthalf[(ob, half)] = t

            load_xb1(0, 0, nc.sync)
            load_mt0c(0, nc.scalar)
            load_xb1(0, 1, nc.gpsimd)
            load_xb1(0, 2, nc.sync)
            load_mt0c(1, nc.scalar)
            load_xb1(0, 3, nc.gpsimd)
            load_xb1(0, 4, nc.sync)
            load_mt0c(2, nc.scalar)
            load_xb1(0, 5, nc.gpsimd)
            load_mth(1, 0, nc.scalar)
            load_xb1(0, 6, nc.sync)
            load_xb1(0, 7, nc.gpsimd)
            load_mt0c(3, nc.scalar)
            load_mth(1, 1, nc.sync)
            # constants + remaining phase-1 weights in consumption order
            nc.gpsimd.dma_start(cp[:], cp_d[:])
            nc.scalar.dma_start(ones[:], ones_d[:])
            load_w(mt, MT_d, 2, "mt", nc.gpsimd)
            load_w(mt, MT_d, 3, "mt", nc.sync)
            load_w(mt, MT_d, 4, "mt", nc.scalar)
            load_w(mt, MT_d, 5, "mt", nc.sync)
            load_w(mt, MT_d, 6, "mt", nc.gpsimd)
            load_w(mt, MT_d, 7, "mt", nc.scalar)
            # x for lb1 (needed ~26us in), then phase-2 weights; xb2/xb3 are
            # issued later from scalar after the lb0/lb1 sigmoids.
            load_xb4(1, 0, nc.gpsimd)
            load_xb4(1, 4, nc.sync)
            load_w(wct, WcT_d, 0, "wct", nc.gpsimd)
            load_w(wct, WcT_d, 1, "wct", nc.sync)
            load_w(wct, WcT_d, 2, "wct", nc.gpsimd)
            load_w(wct, WcT_d, 3, "wct", nc.sync)
            load_w(wct, WcT_d, 4, "wct", nc.gpsimd)
            load_w(wct, WcT_d, 5, "wct", nc.sync)
            load_w(wct, WcT_d, 6, "wct", nc.gpsimd)
            load_w(wct, WcT_d, 7, "wct", nc.sync)

            ub = cp[:, :NH]
            c0b = cp[:, NH:NH + 1]
            bcb = cp[:, NH + 1:NH + 1 + NH]
            bob = cp[:, NH + 1 + NH:]

            def mt_ap(ob, hb):
                if ob == 0:
                    return mt0c[hb // 2][:, (hb % 2) * P:(hb % 2 + 1) * P]
                if (ob, hb // 4) in mthalf:
                    return mthalf[(ob, hb // 4)][:, (hb % 4) * P:(hb % 4 + 1) * P]
                return mt[ob][:, hb * P:(hb + 1) * P]

            sp = [rp.tile([P, 1], dt.float32, name=f"sp{i}", tag=f"sp{i}")
                  for i in range(NL)]

            state = {"pending": None}

            def flush_pending():
                if state["pending"] is None:
                    return
                pd_t, prod_t, lb = state["pending"]
                state["pending"] = None
                nc.tensor.matmul(pd_t[:], ones[:], prod_t[:],
                                 start=True, stop=True)
                ls = slice(lb * LB, (lb + 1) * LB)
                nc.scalar.activation(t_s[:, ls], pd_t[:], AF.Sigmoid,
                                     bias=c0b[:, 0:1], scale=-1.0,
                                     accum_out=sp[lb][:])
                # deferred x prefetch: issue on scalar after the sigmoid so
                # these transfers stay off the wire during the hot start.
                if lb == 0:
                    load_xb4(2, 0, nc.scalar)
                    load_xb4(2, 4, nc.scalar)
                elif lb == 1:
                    load_xb4(3, 0, nc.scalar)
                    load_xb4(3, 4, nc.scalar)

            # ---- phase 1: d and sigmoid path
            for lb in range(NL):
                pd = dp.tile([P, LB], dt.float32)
                acc = None
                for ob in range(NH):
                    py = yp.tile([P, LB], dt.float32, tag="mm")
                    for hb in range(NH):
                        nc.tensor.matmul(
                            py[:], mt_ap(ob, hb), xbt[(lb, hb)],
                            start=(hb == 0), stop=(hb == NH - 1))
                    if ob == 1:
                        flush_pending()
                    prod = wp.tile([P, LB], dt.float32r, tag="prod")
                    nc.vector.scalar_tensor_tensor(
                        prod[:], py[:], ub[:, ob:ob + 1], xbt[(lb, ob)],
                        op0=ALU.add, op1=ALU.mult)
                    if acc is None:
                        acc = prod
                    else:
                        nacc = wp.tile([P, LB], dt.float32r, tag="pacc")
                        nc.vector.tensor_tensor(nacc[:], acc[:], prod[:],
                                                op=ALU.add)
                        acc = nacc
                state["pending"] = (pd, acc, lb)

            # ---- phase 2: out = cs*(Wc x + bc) + bo
            OUT_ENG = [nc.sync, nc.gpsimd]
            oi = 0
            for lb in range(NL):
                for ob in range(NH):
                    if lb == NL - 1 and ob >= NH - 2:
                        nmm = 4
                    else:
                        nmm = 1
                    mw = LB // nmm
                    pzs = []
                    for ck in range(nmm):
                        pz = yp.tile([P, mw], dt.float32, tag="mm")
                        for hb in range(NH):
                            nc.tensor.matmul(
                                pz[:], wct[ob][:, hb * P:(hb + 1) * P],
                                xbt[(lb, hb)][:, ck * mw:(ck + 1) * mw],
                                start=(hb == 0), stop=(hb == NH - 1))
                        pzs.append(pz)
                    if lb == 0 and ob == 0:
                        flush_pending()
                        s01 = rp.tile([P, 1], dt.float32)
                        nc.vector.tensor_tensor(s01[:], sp[0][:], sp[1][:],
                                                op=ALU.add)
                        s23 = rp.tile([P, 1], dt.float32)
                        nc.vector.tensor_tensor(s23[:], sp[2][:], sp[3][:],
                                                op=ALU.add)
                        s_all = rp.tile([P, 1], dt.float32)
                        nc.vector.tensor_tensor(s_all[:], s01[:], s23[:],
                                                op=ALU.add)
                        S1_t = rp.tile([P, 1], dt.float32)
                        nc.vector.tensor_scalar(
                            S1_t[:], s_all[:], 1.0 / (L - 1), 1.0,
                            op0=ALU.mult, op1=ALU.add)
                        nc.vector.tensor_scalar(
                            cs[:], t_s[:], -float(L) / (L - 1), S1_t[:],
                            op0=ALU.mult, op1=ALU.add)
                    for ck in range(nmm):
                        lo = lb * LB + ck * mw
                        lsc = slice(lo, lo + mw)
                        zc = op.tile([P, mw], dt.float32, tag="zc")
                        nc.vector.scalar_tensor_tensor(
                            zc[:], pzs[ck][:], bcb[:, ob:ob + 1], cs[:, lsc],
                            op0=ALU.add, op1=ALU.mult)
                        ot = op.tile([P, mw], dt.bfloat16, tag="ot")
                        if oi % 2 == 1:
                            nc.vector.tensor_scalar_add(
                                ot[:], zc[:], bob[:, ob:ob + 1])
                        else:
                            nc.scalar.activation(ot[:], zc[:], AF.Identity,
                                                 bias=bob[:, ob:ob + 1])
                        # keep gpsimd's DMA queue out of the last few tiles:
                        # whichever engine issues the final DMAs pays a long
                        # queue DRAIN at TileContext exit.
                        if lb == NL - 1 and ob >= NH - 4:
                            eng = [nc.sync, nc.scalar][oi % 2]
                        else:
                            eng = OUT_ENG[oi % len(OUT_ENG)]
                        oi += 1
                        eng.dma_start(out_d[ob * P:(ob + 1) * P, lsc], ot[:])

    nc.compile()
    return nc


def _get_nc():
    if "nc" not in _CACHE:
        _CACHE["nc"] = _build()
    return _CACHE["nc"]


def _prep_inputs(x, Wq, bq, Wk, bk, Wv, bv, Wo, bo):
    f8 = np.float64
    bf = ml_dtypes.bfloat16
    M = (Wq.astype(f8).T @ Wk.astype(f8)).astype(np.float32)
    u = (Wk.astype(f8).T @ bq.astype(f8)
         + Wq.astype(f8).T @ bk.astype(f8)).astype(np.float32)
    c0 = np.float32(bq.astype(f8) @ bk.astype(f8))
    Wc = (Wo.astype(f8) @ Wv.astype(f8)).astype(np.float32)
    bc = (Wo.astype(f8) @ bv.astype(f8)).astype(np.float32)

    def _pack(WT):
        t = WT.reshape(NH, P, NH, P)
        return np.ascontiguousarray(
            t.transpose(2, 1, 0, 3).reshape(NH, P, NH * P).astype(bf))

    MT = _pack(M.T)
    WcT = _pack(Wc.T)
    ub = u.reshape(NH, P).T
    bcb = bc.reshape(NH, P).T
    bob = bo.astype(np.float32).reshape(NH, P).T
    c0b = np.full((P, 1), np.log(L - 1.0) - np.float64(c0), np.float32)
    cpack = np.ascontiguousarray(
        np.concatenate([ub, c0b, bcb, bob], axis=1).astype(np.float32))
    ones = np.ones((P, P), np.float32)

    shared = dict(MT=MT, WcT=WcT, cpack=cpack, ones=ones)
    in_maps = []
    for n in range(N_CORES):
        xT = np.ascontiguousarray(x[n].T.astype(bf))
        in_maps.append(dict(xT=xT, **shared))
    return in_maps


def kernel(x, Wq, bq, Wk, bk, Wv, bv, Wo, bo, _trace=False, _trace_kwargs=None):
    x, Wq, bq, Wk, bk, Wv, bv, Wo, bo = (
        np.asarray(a) for a in (x, Wq, bq, Wk, bk, Wv, bv, Wo, bo))
    nc = _get_nc()
    in_maps = _prep_inputs(x, Wq, bq, Wk, bk, Wv, bv, Wo, bo)
    res = run_bass_kernel_spmd(nc, in_maps, list(range(N_CORES)),
                               trace=_trace, **(_trace_kwargs or {}))
    out = np.empty((N, L, H), np.float32)
    for n in range(N_CORES):
        out[n] = res.results[n]["outT"].T
    if _trace:
        kernel.last_result = res
    return out
